# revision 15
# baseline (speedup 1.0000x reference)
"""Trainium2 Bass kernel for nn_Bilinear (NODE=8192, IN1=IN2=OUT=256).

out[n,o] = sum_{i,j} x1[n,i] * W[o,i,j] * x2[n,j] + b[o]

Strategy (8 NeuronCores, sharded over the O dimension, 32 outputs/core):
  stage 1 (TensorE, fp16): Z[n, (o,j)] = sum_i x1T[i,n] * W[i, (o,j)]
      - lhsT = x1T tile [i=128, n=128] stationary, rhs = W [i=128, (o,j)]
      - accumulate over 2 i-tiles into PSUM [128n, 4096] (16 o's per half)
  stage 2: out[n,o] = sum_j Z[n,o,j] * x2[n,j]
      - ScalarE: cast PSUM fp32 -> SBUF bf16   (G)
      - VectorE: G *= broadcast_o(x2)   (fp16 2x mode)
      - VectorE: 3 pairwise-halving tree levels (bf16 2x) then a
        segmented tensor_reduce (fp32 accum) -> out columns
  The n-tile loop runs as a hardware For_i loop: the static program is
  ~60 instructions (static-instruction overhead dominates in this env).

Host side: shard W over cores, pre-transpose x1 -> x1T and
W -> [I, (o,j)] layout, cast inputs to fp16, add bias after gather.
"""
import os
import sys

for _p in ("/opt/trn_rl_repo", "/root/.axon_site/_ro/trn_rl_repo"):
    if _p not in sys.path and os.path.isdir(_p):
        sys.path.append(_p)

import numpy as np
import ml_dtypes

import concourse.bass as bass
import concourse.mybir as mybir
import concourse.tile as tile
from concourse import bass_utils

NODE, IN1, IN2, OUT = 8192, 256, 256, 256
N_CORES = 8
O_SHARD = OUT // N_CORES  # 32 outputs per core

F32 = mybir.dt.float32
F16 = mybir.dt.float16

N_TILES = NODE // 128          # 64 n-tiles
HALF_O = O_SHARD // 2          # 16 o's per half (4096 cols)


def _split_multiwait_insts(nc):
    """This walrus build only supports one sem-wait per instruction for
    several instruction structs. Split any multi-wait instruction into
    single-wait NoOps + the original instruction with one wait."""
    n_fixed = 0
    for fn in nc.m.functions:
        for bb in fn.blocks:
            insts = bb.instructions
            i = 0
            while i < len(insts):
                inst = insts[i]
                si = getattr(inst, "sync_info", None)
                if si is not None and si.on_wait and len(si.on_wait) > 1:
                    waits = list(si.on_wait)
                    new_nops = []
                    for k, w in enumerate(waits[:-1]):
                        nop = mybir.InstNoOp(
                            name=f"{inst.name}-wsplit{k}",
                            engine=inst.engine,
                            ins=[],
                            outs=[],
                            sync_info=mybir.SyncInfo(on_wait=[w], on_update=[]),
                        )
                        new_nops.append(nop)
                    inst.sync_info = mybir.SyncInfo(
                        on_wait=[waits[-1]], on_update=list(si.on_update or [])
                    )
                    for k, nop in enumerate(new_nops):
                        insts.insert(i + k, nop)
                    i += len(new_nops)
                    n_fixed += 1
                i += 1
    return n_fixed


def build_nc(reps: int = 1, staggered: bool = True):
    nc = bass.Bass("TRN2", target_bir_lowering=False, debug=False)
    # sharded inputs arrive as ONE uint8 blob per core (single big host->
    # device transfer is markedly faster than three): W shard (natural
    # [o,i,j] f16), then 1/8 of x1T (by i-rows), then 1/8 of x2 (by nodes);
    # full x1T/x2 are assembled on-device via AllGather.
    W_BYTES = O_SHARD * IN1 * IN2 * 2
    X1S_BYTES = (IN1 // N_CORES) * NODE * 2
    X2S_BYTES = (NODE // N_CORES) * IN2 * 2
    BLOB_BYTES = W_BYTES + X1S_BYTES + X2S_BYTES
    blob = nc.dram_tensor("blob", [BLOB_BYTES], mybir.dt.uint8, kind="ExternalInput").ap()
    wt = blob[0:W_BYTES].bitcast(F16).rearrange(
        "(o i j) -> o i j", o=O_SHARD, i=IN1
    )
    x1ts = blob[W_BYTES : W_BYTES + X1S_BYTES].bitcast(F16).rearrange(
        "(a n) -> a n", n=NODE
    )
    x2s = blob[W_BYTES + X1S_BYTES : BLOB_BYTES].bitcast(F16).rearrange(
        "(a j) -> a j", j=IN2
    )
    out = nc.dram_tensor("out", [NODE, O_SHARD], F16, kind="ExternalOutput").ap()

    x1i = nc.dram_tensor("x1i", [IN1 // N_CORES, NODE], F16).ap()
    x2i = nc.dram_tensor("x2i", [NODE // N_CORES, IN2], F16).ap()
    x1t = nc.dram_tensor("x1g", [IN1, NODE], F16, addr_space="Shared").ap()
    x2b = nc.dram_tensor("x2g", [NODE, IN2], F16, addr_space="Shared").ap()

    x2_src = x2b.rearrange("(t p) j -> p t j", p=128)  # [128, 64, 256]

    with tile.TileContext(nc) as tc:
        with (
            tc.tile_pool(name="wp", bufs=1) as wp,
            tc.tile_pool(name="x1p", bufs=2) as x1p,
            tc.tile_pool(name="x2p", bufs=1) as x2p,
            tc.tile_pool(name="ps", bufs=1, space="PSUM") as psp,
            tc.tile_pool(name="gp", bufs=2) as gp,
            tc.tile_pool(name="tp", bufs=2) as tp,
            tc.tile_pool(name="op", bufs=2) as op,
        ):
            from contextlib import nullcontext

            # assemble full x1T / x2 on device (outside the rep loop:
            # collectives inside a For_i wedge the device)
            nc.sync.dma_start(x1i[:, :], x1ts[:, :])
            nc.sync.dma_start(x2i[:, :], x2s[:, :])
            nc.gpsimd.collective_compute(
                "AllGather",
                mybir.AluOpType.bypass,
                ins=[x1i[:, :]],
                outs=[x1t[:, :]],
                replica_groups=[list(range(N_CORES))],
            )
            nc.gpsimd.collective_compute(
                "AllGather",
                mybir.AluOpType.bypass,
                ins=[x2i[:, :]],
                outs=[x2b[:, :]],
                replica_groups=[list(range(N_CORES))],
            )
            rep_ctx = tc.For_i(0, reps, 1) if reps > 1 else nullcontext()
            with rep_ctx:
                # resident inputs; W arrives in natural [o, i, j] layout and
                # is rearranged to [i-partition, (o, j)] by the load DMA's AP
                w_sb = []
                for it in range(2):
                    w_t = wp.tile([128, O_SHARD * IN2], F16, tag=f"w{it}")
                    nc.sync.dma_start(
                        w_t[:, :].rearrange("p (o j) -> p o j", j=IN2),
                        wt[:, it * 128 : (it + 1) * 128, :].rearrange(
                            "o p j -> p o j"
                        ),
                    )
                    w_sb.append(w_t)
                x2_sb = x2p.tile([128, N_TILES * IN2], F16, tag="x2")
                nc.sync.dma_start(
                    x2_sb[:, :].rearrange("p (t j) -> p t j", j=IN2), x2_src
                )

                # hardware loop over n-tiles; iv = node offset (t*128)
                with tc.For_i(0, NODE, 128, staggered_reset=staggered) as iv:
                    # stream this n-tile of x1T (stationary operands need
                    # static SBUF offsets, so DMA into fixed tiles)
                    x1_cur = []
                    for it in range(2):
                        x1_t = x1p.tile([128, 128], F16, tag=f"x1c{it}")
                        nc.sync.dma_start(
                            x1_t[:, :],
                            x1t[it * 128 : (it + 1) * 128, bass.ds(iv, 128)],
                        )
                        x1_cur.append(x1_t)
                    out_t = op.tile([128, O_SHARD], F16, tag="out")
                    for half in range(2):
                        ps = psp.tile([128, HALF_O * IN2], F32, tag="ps")
                        for it in range(2):
                            lhs = x1_cur[it][:, :]
                            for m in range(8):
                                col0 = half * HALF_O * IN2 + m * 512
                                nc.tensor.matmul(
                                    ps[:, m * 512 : (m + 1) * 512],
                                    lhs,
                                    w_sb[it][:, col0 : col0 + 512],
                                    start=(it == 0),
                                    stop=(it == 1),
                                )
                        g = gp.tile([128, HALF_O * IN2], F16, tag="g")
                        # cast fp32 PSUM -> bf16 SBUF (ScalarE)
                        nc.scalar.copy(g[:, :], ps[:, :])
                        # multiply by broadcast x2 (VectorE fp16 2x), in place
                        gv = g[:, :].rearrange("p (o j) -> p o j", o=HALF_O)
                        x2t = x2_sb[:, bass.ds(iv * 2, IN2)]  # [128, 256] (t*256)
                        nc.vector.tensor_tensor(
                            gv,
                            gv,
                            x2t[:, None, :].broadcast_to([128, HALF_O, IN2]),
                            mybir.AluOpType.mult,
                        )
                        # 3 fp16 tree levels (2x mode), then fp32 seg-reduce
                        cur = gv
                        width = IN2
                        for _lvl in range(3):
                            hw_ = width // 2
                            nxt = tp.tile([128, HALF_O, hw_], F16, tag=f"t{hw_}")
                            nc.vector.tensor_tensor(
                                nxt[:, :, :],
                                cur[:, :, 0:hw_],
                                cur[:, :, hw_:width],
                                mybir.AluOpType.add,
                            )
                            cur = nxt
                            width = hw_
                        with nc.allow_low_precision("fp16 output requested"):
                            nc.vector.tensor_reduce(
                                out_t[:, half * HALF_O : (half + 1) * HALF_O],
                                cur,
                                mybir.AxisListType.X,
                                mybir.AluOpType.add,
                            )
                    nc.sync.dma_start(out[bass.ds(iv, 128), :], out_t[:, :])

    _split_multiwait_insts(nc)
    return nc


_NC_CACHE = {}


def _get_nc(reps: int = 1):
    if reps not in _NC_CACHE:
        _NC_CACHE[reps] = build_nc(reps)
    return _NC_CACHE[reps]


def _make_in_maps(x1, x2, weight):
    x1 = np.asarray(x1, dtype=np.float32)
    x2 = np.asarray(x2, dtype=np.float32)
    weight = np.asarray(weight, dtype=np.float32)
    x1t = np.ascontiguousarray(x1.T.astype(np.float16))  # [IN1, NODE]
    x2b = np.ascontiguousarray(x2.astype(np.float16))
    ri = IN1 // N_CORES
    rn = NODE // N_CORES
    in_maps = []
    w16 = weight.astype(np.float16)  # natural [O, I, J] layout
    for c in range(N_CORES):
        blob = np.concatenate(
            [
                w16[c * O_SHARD : (c + 1) * O_SHARD].ravel().view(np.uint8),
                x1t[c * ri : (c + 1) * ri, :].ravel().view(np.uint8),
                x2b[c * rn : (c + 1) * rn, :].ravel().view(np.uint8),
            ]
        )
        in_maps.append({"blob": blob})
    return in_maps


def run_on_device(x1, x2, weight, reps: int = 1):
    nc = _get_nc(reps)
    in_maps = _make_in_maps(x1, x2, weight)
    res = bass_utils.run_bass_kernel_spmd(nc, in_maps, core_ids=list(range(N_CORES)))
    out = np.concatenate(
        [res.results[c]["out"].astype(np.float32) for c in range(N_CORES)], axis=1
    )
    return out


def kernel(x1, x2, weight, bias):
    out = run_on_device(x1, x2, weight, reps=1)
    bias = np.asarray(bias, dtype=np.float32)
    return (out + bias[None, :]).astype(np.float32)


def _warmup():
    """Build + compile the NEFF and prime the jit/device at import time so
    the first kernel() call pays only transfer + execution."""
    try:
        z1 = np.zeros((NODE, IN1), dtype=np.float32)
        z2 = np.zeros((NODE, IN2), dtype=np.float32)
        zw = np.zeros((OUT, IN1, IN2), dtype=np.float32)
        run_on_device(z1, z2, zw, reps=1)
    except Exception:
        # defer any environment problem to the real kernel() call
        _NC_CACHE.clear()


if os.environ.get("BILINEAR_KERNEL_NO_WARMUP", "") != "1":
    _warmup()


if __name__ == "__main__":
    rng = np.random.default_rng(0)
    x1 = rng.standard_normal((NODE, IN1), dtype=np.float32)
    x2 = rng.standard_normal((NODE, IN2), dtype=np.float32)
    w = (rng.uniform(-1, 1, size=(OUT, IN1, IN2)) / 256.0).astype(np.float32)
    b = np.zeros(OUT, dtype=np.float32)
    got = kernel(x1, x2, w, b)
    print("out shape", got.shape, got.dtype)


# revision 16
# speedup vs baseline: 1.5642x; 1.5642x over previous
"""Trainium2 Bass kernel for nn_Bilinear (NODE=8192, IN1=IN2=OUT=256).

out[n,o] = sum_{i,j} x1[n,i] * W[o,i,j] * x2[n,j] + b[o]

Strategy (8 NeuronCores, sharded over the O dimension, 32 outputs/core):
  stage 1 (TensorE, fp16): Z[n, (o,j)] = sum_i x1T[i,n] * W[i, (o,j)]
      - lhsT = x1T tile [i=128, n=128] stationary, rhs = W [i=128, (o,j)]
      - accumulate over 2 i-tiles into PSUM [128n, 4096] (16 o's per half)
  stage 2: out[n,o] = sum_j Z[n,o,j] * x2[n,j]
      - ScalarE: cast PSUM fp32 -> SBUF bf16   (G)
      - VectorE: G *= broadcast_o(x2)   (fp16 2x mode)
      - VectorE: 3 pairwise-halving tree levels (bf16 2x) then a
        segmented tensor_reduce (fp32 accum) -> out columns
  The n-tile loop runs as a hardware For_i loop: the static program is
  ~60 instructions (static-instruction overhead dominates in this env).

Host side: shard W over cores, pre-transpose x1 -> x1T and
W -> [I, (o,j)] layout, cast inputs to fp16, add bias after gather.
"""
import os
import sys

for _p in ("/opt/trn_rl_repo", "/root/.axon_site/_ro/trn_rl_repo"):
    if _p not in sys.path and os.path.isdir(_p):
        sys.path.append(_p)

import numpy as np
import ml_dtypes

import concourse.bass as bass
import concourse.mybir as mybir
import concourse.tile as tile
from concourse import bass_utils

NODE, IN1, IN2, OUT = 8192, 256, 256, 256
N_CORES = 8
O_SHARD = OUT // N_CORES  # 32 outputs per core

F32 = mybir.dt.float32
F16 = mybir.dt.float16

N_TILES = NODE // 128          # 64 n-tiles
HALF_O = O_SHARD // 2          # 16 o's per half (4096 cols)


def _split_multiwait_insts(nc):
    """This walrus build only supports one sem-wait per instruction for
    several instruction structs. Split any multi-wait instruction into
    single-wait NoOps + the original instruction with one wait."""
    n_fixed = 0
    for fn in nc.m.functions:
        for bb in fn.blocks:
            insts = bb.instructions
            i = 0
            while i < len(insts):
                inst = insts[i]
                si = getattr(inst, "sync_info", None)
                if si is not None and si.on_wait and len(si.on_wait) > 1:
                    waits = list(si.on_wait)
                    new_nops = []
                    for k, w in enumerate(waits[:-1]):
                        nop = mybir.InstNoOp(
                            name=f"{inst.name}-wsplit{k}",
                            engine=inst.engine,
                            ins=[],
                            outs=[],
                            sync_info=mybir.SyncInfo(on_wait=[w], on_update=[]),
                        )
                        new_nops.append(nop)
                    inst.sync_info = mybir.SyncInfo(
                        on_wait=[waits[-1]], on_update=list(si.on_update or [])
                    )
                    for k, nop in enumerate(new_nops):
                        insts.insert(i + k, nop)
                    i += len(new_nops)
                    n_fixed += 1
                i += 1
    return n_fixed


def build_nc(reps: int = 1, staggered: bool = True):
    nc = bass.Bass("TRN2", target_bir_lowering=False, debug=False)
    # sharded inputs: each core receives 1/8 of x1T (by i-rows) and 1/8 of
    # x2 (by nodes); full tensors are assembled on-device via AllGather.
    x1ts = nc.dram_tensor("x1ts", [IN1 // N_CORES, NODE], F16, kind="ExternalInput").ap()
    x2s = nc.dram_tensor("x2s", [NODE // N_CORES, IN2], F16, kind="ExternalInput").ap()
    wt = nc.dram_tensor("wt", [O_SHARD, IN1, IN2], F16, kind="ExternalInput").ap()
    out = nc.dram_tensor("out", [NODE, O_SHARD], F16, kind="ExternalOutput").ap()

    x1i = nc.dram_tensor("x1i", [IN1 // N_CORES, NODE], F16).ap()
    x2i = nc.dram_tensor("x2i", [NODE // N_CORES, IN2], F16).ap()
    x1t = nc.dram_tensor("x1g", [IN1, NODE], F16, addr_space="Shared").ap()
    x2b = nc.dram_tensor("x2g", [NODE, IN2], F16, addr_space="Shared").ap()

    x2_src = x2b.rearrange("(t p) j -> p t j", p=128)  # [128, 64, 256]

    with tile.TileContext(nc) as tc:
        with (
            tc.tile_pool(name="wp", bufs=1) as wp,
            tc.tile_pool(name="x1p", bufs=2) as x1p,
            tc.tile_pool(name="x2p", bufs=1) as x2p,
            tc.tile_pool(name="ps", bufs=1, space="PSUM") as psp,
            tc.tile_pool(name="gp", bufs=2) as gp,
            tc.tile_pool(name="tp", bufs=2) as tp,
            tc.tile_pool(name="op", bufs=2) as op,
        ):
            from contextlib import nullcontext

            # assemble full x1T / x2 on device (outside the rep loop:
            # collectives inside a For_i wedge the device)
            nc.sync.dma_start(x1i[:, :], x1ts[:, :])
            nc.sync.dma_start(x2i[:, :], x2s[:, :])
            nc.gpsimd.collective_compute(
                "AllGather",
                mybir.AluOpType.bypass,
                ins=[x1i[:, :]],
                outs=[x1t[:, :]],
                replica_groups=[list(range(N_CORES))],
            )
            nc.gpsimd.collective_compute(
                "AllGather",
                mybir.AluOpType.bypass,
                ins=[x2i[:, :]],
                outs=[x2b[:, :]],
                replica_groups=[list(range(N_CORES))],
            )
            rep_ctx = tc.For_i(0, reps, 1) if reps > 1 else nullcontext()
            with rep_ctx:
                # resident inputs; W arrives in natural [o, i, j] layout and
                # is rearranged to [i-partition, (o, j)] by the load DMA's AP
                w_sb = []
                for it in range(2):
                    w_t = wp.tile([128, O_SHARD * IN2], F16, tag=f"w{it}")
                    nc.sync.dma_start(
                        w_t[:, :].rearrange("p (o j) -> p o j", j=IN2),
                        wt[:, it * 128 : (it + 1) * 128, :].rearrange(
                            "o p j -> p o j"
                        ),
                    )
                    w_sb.append(w_t)
                x2_sb = x2p.tile([128, N_TILES * IN2], F16, tag="x2")
                nc.sync.dma_start(
                    x2_sb[:, :].rearrange("p (t j) -> p t j", j=IN2), x2_src
                )

                # hardware loop over n-tiles; iv = node offset (t*128)
                with tc.For_i(0, NODE, 128, staggered_reset=staggered) as iv:
                    # stream this n-tile of x1T (stationary operands need
                    # static SBUF offsets, so DMA into fixed tiles)
                    x1_cur = []
                    for it in range(2):
                        x1_t = x1p.tile([128, 128], F16, tag=f"x1c{it}")
                        nc.sync.dma_start(
                            x1_t[:, :],
                            x1t[it * 128 : (it + 1) * 128, bass.ds(iv, 128)],
                        )
                        x1_cur.append(x1_t)
                    out_t = op.tile([128, O_SHARD], F16, tag="out")
                    for half in range(2):
                        ps = psp.tile([128, HALF_O * IN2], F32, tag="ps")
                        for it in range(2):
                            lhs = x1_cur[it][:, :]
                            for m in range(8):
                                col0 = half * HALF_O * IN2 + m * 512
                                nc.tensor.matmul(
                                    ps[:, m * 512 : (m + 1) * 512],
                                    lhs,
                                    w_sb[it][:, col0 : col0 + 512],
                                    start=(it == 0),
                                    stop=(it == 1),
                                )
                        g = gp.tile([128, HALF_O * IN2], F16, tag="g")
                        # cast fp32 PSUM -> bf16 SBUF (ScalarE)
                        nc.scalar.copy(g[:, :], ps[:, :])
                        # multiply by broadcast x2 (VectorE fp16 2x), in place
                        gv = g[:, :].rearrange("p (o j) -> p o j", o=HALF_O)
                        x2t = x2_sb[:, bass.ds(iv * 2, IN2)]  # [128, 256] (t*256)
                        nc.vector.tensor_tensor(
                            gv,
                            gv,
                            x2t[:, None, :].broadcast_to([128, HALF_O, IN2]),
                            mybir.AluOpType.mult,
                        )
                        # 3 fp16 tree levels (2x mode), then fp32 seg-reduce
                        cur = gv
                        width = IN2
                        for _lvl in range(3):
                            hw_ = width // 2
                            nxt = tp.tile([128, HALF_O, hw_], F16, tag=f"t{hw_}")
                            nc.vector.tensor_tensor(
                                nxt[:, :, :],
                                cur[:, :, 0:hw_],
                                cur[:, :, hw_:width],
                                mybir.AluOpType.add,
                            )
                            cur = nxt
                            width = hw_
                        with nc.allow_low_precision("fp16 output requested"):
                            nc.vector.tensor_reduce(
                                out_t[:, half * HALF_O : (half + 1) * HALF_O],
                                cur,
                                mybir.AxisListType.X,
                                mybir.AluOpType.add,
                            )
                    nc.sync.dma_start(out[bass.ds(iv, 128), :], out_t[:, :])

    _split_multiwait_insts(nc)
    return nc


_NC_CACHE = {}


def _get_nc(reps: int = 1):
    if reps not in _NC_CACHE:
        _NC_CACHE[reps] = build_nc(reps)
    return _NC_CACHE[reps]


def _make_in_maps(x1, x2, weight):
    x1 = np.asarray(x1, dtype=np.float32)
    x2 = np.asarray(x2, dtype=np.float32)
    weight = np.asarray(weight, dtype=np.float32)
    x1t = np.ascontiguousarray(x1.T.astype(np.float16))  # [IN1, NODE]
    x2b = np.ascontiguousarray(x2.astype(np.float16))
    ri = IN1 // N_CORES
    rn = NODE // N_CORES
    in_maps = []
    w16 = weight.astype(np.float16)  # natural [O, I, J] layout
    for c in range(N_CORES):
        wt = np.ascontiguousarray(w16[c * O_SHARD : (c + 1) * O_SHARD])
        in_maps.append(
            {
                "x1ts": np.ascontiguousarray(x1t[c * ri : (c + 1) * ri, :]),
                "x2s": np.ascontiguousarray(x2b[c * rn : (c + 1) * rn, :]),
                "wt": wt,
            }
        )
    return in_maps


def run_on_device(x1, x2, weight, reps: int = 1):
    nc = _get_nc(reps)
    in_maps = _make_in_maps(x1, x2, weight)
    res = bass_utils.run_bass_kernel_spmd(nc, in_maps, core_ids=list(range(N_CORES)))
    out = np.concatenate(
        [res.results[c]["out"].astype(np.float32) for c in range(N_CORES)], axis=1
    )
    return out


def kernel(x1, x2, weight, bias):
    out = run_on_device(x1, x2, weight, reps=1)
    bias = np.asarray(bias, dtype=np.float32)
    return (out + bias[None, :]).astype(np.float32)


def _warmup():
    """Build + compile the NEFF and prime the jit/device at import time so
    the first kernel() call pays only transfer + execution."""
    try:
        z1 = np.zeros((NODE, IN1), dtype=np.float32)
        z2 = np.zeros((NODE, IN2), dtype=np.float32)
        zw = np.zeros((OUT, IN1, IN2), dtype=np.float32)
        run_on_device(z1, z2, zw, reps=1)
    except Exception:
        # defer any environment problem to the real kernel() call
        _NC_CACHE.clear()


if os.environ.get("BILINEAR_KERNEL_NO_WARMUP", "") != "1":
    _warmup()


if __name__ == "__main__":
    rng = np.random.default_rng(0)
    x1 = rng.standard_normal((NODE, IN1), dtype=np.float32)
    x2 = rng.standard_normal((NODE, IN2), dtype=np.float32)
    w = (rng.uniform(-1, 1, size=(OUT, IN1, IN2)) / 256.0).astype(np.float32)
    b = np.zeros(OUT, dtype=np.float32)
    got = kernel(x1, x2, w, b)
    print("out shape", got.shape, got.dtype)
